# revision 4
# baseline (speedup 1.0000x reference)
"""Trainium2 Bass kernel for nn_NewSSHA_6528350290307.

Model (see reference): per-head K/Q projections over full d_model, unscaled
attention scores[i,j] = k_i . q_j softmaxed over j, heads = attn @ x, concat
heads -> att_linear -> relu MLP.

Sharding: 8 cores = 4 batches x 2 query-row halves. Each core owns 1024
output rows end-to-end: it computes q projections for its whole batch
(j side), k projections for its 1024 query rows, per-head attention, and
folds the head-concat @ W_att into a sum over heads of
head_h @ W_att[h*512:(h+1)*512], so no cross-core communication is needed.
The host rotates each batch so a core's query rows are always rows 0:1024
of its input (attention over keys j is permutation invariant).

Matmuls run as float32r (fp32 bits, PE rounds to ~13-bit mantissa,
1 cycle/row instead of fp32's 4). Transposes and the softmax denominator
pipeline stay exact fp32.
"""

import sys

sys.path.insert(0, "/opt/trn_rl_repo")

import numpy as np

import concourse.bass as bass
import concourse.mybir as mybir
import concourse.tile as tile

F32 = mybir.dt.float32
F32R = mybir.dt.float32r

B, N, D, H, DFF = 4, 2048, 512, 8, 2048
NQ = N // 2          # query rows per core
JT = N // 128        # 16 key tiles of 128 rows
IB = NQ // 512       # 2 i-blocks of 512 query rows
DC = D // 128        # 4 chunks of d_model
GC = DFF // 128      # 16 chunks of d_ff
T8 = NQ // 128       # 8 query chunks of 128
EXP = mybir.ActivationFunctionType.Exp
RELU = mybir.ActivationFunctionType.Relu

# ---------------------------------------------------------------------------
# Workarounds for the 1-sync-wait-per-instruction cap in this walrus build.
# ---------------------------------------------------------------------------


def _apply_patches():
    import re

    import orjson

    import concourse.bass2jax as bass2jax
    import concourse.bass_utils as bass_utils
    from concourse.tile_scheduler import is_hwdge_proc, is_swdge_proc

    if getattr(tile.TileContext, "_ssha_patched", False):
        return

    # 1. Tail drain: the stock exit stacks one wait per live proc sem on a
    #    single Drain instruction; split into individual wait_ge nops.
    def _drain_and_barrier(self, tick_clock, wait_clock):
        nc = self.nc
        gc = tick_clock.global_clock
        try:
            ticks = [gc[p] for p in range(27)]
        except TypeError:
            ticks = [int(s) for s in re.findall(r"\d+", repr(gc))]
        alloc = wait_clock.sems.allocated()
        for proc, sem in sorted(alloc.items()):
            t = ticks[proc] if proc < len(ticks) else 0
            if t <= 0:
                continue
            val = t * 16 if (is_swdge_proc(proc) or is_hwdge_proc(proc)) else t
            nc.sync.wait_ge(sem, val)
        nc.sync.drain()
        nc.all_engine_barrier()
        assert self.sems is not None
        popped = nc._tile_sem_poison_stack.pop()
        assert popped is self._sem_poison
        nc.clear_and_free_semaphores(list(self.sems.allocated().values()))
        nc.all_engine_barrier()

    tile.TileContext._drain_and_barrier = _drain_and_barrier
    tile.TileContext._ssha_patched = True

    # 2. Tile can still attach 2 waits to one instruction (e.g. self-loading
    #    fp32/f32r matmuls whose codegen has a single S3_LW sync slot).
    #    Post-process the BIR JSON: spill all but one wait per instruction
    #    into EventSemaphore instructions inserted right before it.
    engine_like = {"PE", "DVE", "Activation", "Pool", "SP"}
    counter = [0]

    def _cap_waits(bir_json):
        j = orjson.loads(bir_json)
        changed = False
        for fn in j.get("functions", []):
            for blk in fn.get("blocks", []):
                insts = blk.get("instructions")
                if not insts:
                    continue
                out = []
                for ins in insts:
                    si = ins.get("sync_info")
                    ow = (si or {}).get("on_wait") or []
                    if len(ow) > 1:
                        eng = ins.get("engine")
                        assert eng in engine_like, (
                            f"{ins.get('name')} on {eng} has {len(ow)} waits"
                        )
                        for w in ow[:-1]:
                            counter[0] += 1
                            out.append({
                                "debug": ins.get("debug", 0),
                                "engine": eng,
                                "ins": [],
                                "name": f"I-wspill-{counter[0]}",
                                "opcode": "EventSemaphore",
                                "outs": [],
                                "sync_info": {"on_update": [], "on_wait": [w]},
                            })
                        si["on_wait"] = ow[-1:]
                        changed = True
                    out.append(ins)
                blk["instructions"] = out
        return orjson.dumps(j) if changed else bir_json

    orig_compile = bass_utils.compile_bir_kernel

    def _patched_compile(bir_json, tmpdir, neff_name="file.neff"):
        if isinstance(bir_json, str):
            bir_json = bir_json.encode()
        return orig_compile(_cap_waits(bir_json), tmpdir, neff_name)

    bass_utils.compile_bir_kernel = _patched_compile
    bass2jax.compile_bir_kernel = _patched_compile


# ---------------------------------------------------------------------------
# Kernel builder
# ---------------------------------------------------------------------------


def build_nc():
    from concourse.masks import make_identity

    nc = bass.Bass(dynamic_dma_scratch_size=2048)

    xb_d = nc.declare_dram_parameter("xb", [N, D], F32, isOutput=False)
    wk_d = nc.declare_dram_parameter("wk", [H, D, D], F32, isOutput=False)
    wq_d = nc.declare_dram_parameter("wq", [H, D, D], F32, isOutput=False)
    bk_d = nc.declare_dram_parameter("bk", [H, D], F32, isOutput=False)
    bq_d = nc.declare_dram_parameter("bq", [H, D], F32, isOutput=False)
    wa_d = nc.declare_dram_parameter("watt", [H, D, D], F32, isOutput=False)
    ba_d = nc.declare_dram_parameter("batt", [D], F32, isOutput=False)
    w1_d = nc.declare_dram_parameter("w1", [D, DFF], F32, isOutput=False)
    b1_d = nc.declare_dram_parameter("b1", [DFF], F32, isOutput=False)
    w2_d = nc.declare_dram_parameter("w2", [DFF, D], F32, isOutput=False)
    b2_d = nc.declare_dram_parameter("b2", [D], F32, isOutput=False)
    out_d = nc.declare_dram_parameter("out", [NQ, D], F32, isOutput=True)

    def bcast_ap(vec_ap, parts=128):
        # [n] dram vector -> [parts, n] with partition step 0
        return bass.AP(
            tensor=vec_ap.tensor,
            offset=vec_ap.offset,
            ap=[[0, parts]] + list(vec_ap.ap),
        )

    with tile.TileContext(nc) as tc:
        with (
            tc.tile_pool(name="const", bufs=1) as constp,
            tc.tile_pool(name="yacc", bufs=8) as yaccp,
            tc.tile_pool(name="small", bufs=1) as smallp,
            tc.tile_pool(name="tmp", bufs=2) as tmpp,
            tc.tile_pool(name="ps_mm", bufs=3, space="PSUM") as ps_mm,
        ):
            ident = constp.tile([128, 128], F32)
            make_identity(nc, ident)
            ones_f = constp.tile([128, 1], F32)
            nc.vector.memset(ones_f, 1.0)
            ones_r = constp.tile([128, 1], F32R)
            nc.vector.tensor_copy(ones_r[:], ones_f[:])
            # biases as per-partition columns
            bk_t = constp.tile([128, H, DC], F32)
            nc.sync.dma_start(bk_t[:], bk_d.rearrange("h (c p) -> p h c", p=128))
            bq_t = constp.tile([128, H, DC], F32)
            nc.sync.dma_start(bq_t[:], bq_d.rearrange("h (c p) -> p h c", p=128))
            b1_t = constp.tile([128, GC], F32)
            nc.sync.dma_start(b1_t[:], b1_d.rearrange("(c p) -> p c", p=128))
            # broadcast b_att / b2 across partitions
            ba_bc = constp.tile([128, D], F32)
            nc.sync.dma_start(ba_bc[:], bcast_ap(ba_d[:]))

            y_acc = [yaccp.tile([128, D], F32, tag="yacc", name=f"yacc{t}")
                      for t in range(T8)]

            with (
                tc.tile_pool(name="xp", bufs=1) as xp,
                tc.tile_pool(name="xtp", bufs=DC) as xtp,
                tc.tile_pool(name="qkp", bufs=DC) as qkp,
                tc.tile_pool(name="wp", bufs=2) as wp,
                tc.tile_pool(name="up", bufs=JT) as up,
                tc.tile_pool(name="hsp", bufs=DC) as hsp,
                tc.tile_pool(name="ps_sc", bufs=2, space="PSUM") as ps_sc,
                tc.tile_pool(name="ps_ht", bufs=2, space="PSUM") as ps_ht,
                tc.tile_pool(name="ps_dn", bufs=1, space="PSUM") as ps_dn,
            ):
                # ---- stage A: load x (rows layout, f32r bits) ----
                X = xp.tile([128, JT, D], F32R, tag="X")
                nc.sync.dma_start(
                    X[:], xb_d.rearrange("(rt p) d -> p rt d", p=128).bitcast(F32R)
                )

                # ---- stage B: xT[d, j] via PE transposes ----
                xT = [xtp.tile([128, N], F32R, tag="xT", name=f"xT{d}")
                      for d in range(DC)]
                for rt in range(JT):
                    for dc in range(DC):
                        tp = ps_mm.tile([128, 128], F32, tag="mm")
                        nc.tensor.transpose(
                            tp[:],
                            X[:, rt, dc * 128:(dc + 1) * 128].bitcast(F32),
                            ident[:],
                        )
                        nc.vector.tensor_copy(
                            xT[dc][:, rt * 128:(rt + 1) * 128], tp[:]
                        )

                # ---- stage C: heads ----
                for h in range(H):
                    wk_sb = wp.tile([128, DC, D], F32R, tag="wkq")
                    nc.sync.dma_start(
                        wk_sb[:],
                        wk_d[h].rearrange("(c p) f -> p c f", p=128).bitcast(F32R),
                    )
                    wq_sb = wp.tile([128, DC, D], F32R, tag="wkq")
                    nc.sync.dma_start(
                        wq_sb[:],
                        wq_d[h].rearrange("(c p) f -> p c f", p=128).bitcast(F32R),
                    )
                    wa_sb = wp.tile([128, DC, D], F32R, tag="wa")
                    nc.sync.dma_start(
                        wa_sb[:],
                        wa_d[h].rearrange("(c p) f -> p c f", p=128).bitcast(F32R),
                    )

                    # projections: qT[f, j] (all j), kT[f, i] (my 1024 rows)
                    qT = [qkp.tile([128, N], F32R, tag="qT", name=f"qT{d}")
                          for d in range(DC)]
                    kT = [qkp.tile([128, NQ], F32R, tag="kT", name=f"kT{d}")
                          for d in range(DC)]
                    for fc in range(DC):
                        for jb in range(N // 512):
                            pp = ps_mm.tile([128, 512], F32, tag="mm")
                            for kc in range(DC):
                                nc.tensor.matmul(
                                    pp[:],
                                    wq_sb[:, kc, fc * 128:(fc + 1) * 128],
                                    xT[kc][:, jb * 512:(jb + 1) * 512],
                                    start=(kc == 0),
                                    stop=(kc == DC - 1),
                                )
                            nc.vector.tensor_scalar_add(
                                qT[fc][:, jb * 512:(jb + 1) * 512],
                                pp[:],
                                bq_t[:, h, fc:fc + 1],
                            )
                        for ibk in range(NQ // 512):
                            pp = ps_mm.tile([128, 512], F32, tag="mm")
                            for kc in range(DC):
                                nc.tensor.matmul(
                                    pp[:],
                                    wk_sb[:, kc, fc * 128:(fc + 1) * 128],
                                    xT[kc][:, ibk * 512:(ibk + 1) * 512],
                                    start=(kc == 0),
                                    stop=(kc == DC - 1),
                                )
                            nc.vector.tensor_scalar_add(
                                kT[fc][:, ibk * 512:(ibk + 1) * 512],
                                pp[:],
                                bk_t[:, h, fc:fc + 1],
                            )

                    for ib in range(IB):
                        isl = slice(ib * 512, (ib + 1) * 512)
                        # scores^T -> exp -> denominator, per key tile jt
                        u = []
                        dn_ps = ps_dn.tile([1, 512], F32, tag="dn")
                        sc_prev = None
                        for jt in range(JT):
                            sc = ps_sc.tile([128, 512], F32, tag="sc")
                            for fc in range(DC):
                                nc.tensor.matmul(
                                    sc[:],
                                    qT[fc][:, jt * 128:(jt + 1) * 128],
                                    kT[fc][:, isl],
                                    start=(fc == 0),
                                    stop=(fc == DC - 1),
                                )
                            ut = up.tile([128, 512], F32R, tag="u")
                            nc.scalar.activation(ut[:], sc[:], EXP)
                            u.append(ut)
                            # stagger the denominator matmul one tile behind so
                            # the PE never waits on the exp it just requested
                            if jt >= 1:
                                nc.tensor.matmul(
                                    dn_ps[:], ones_r[:], u[jt - 1][:],
                                    start=(jt == 1), stop=False,
                                )
                        nc.tensor.matmul(
                            dn_ps[:], ones_r[:], u[JT - 1][:],
                            start=False, stop=True,
                        )

                        # attn @ x : headsT_un[d, i] accumulated over j
                        hs = []
                        for dc in range(DC):
                            ht = ps_ht.tile([128, 512], F32, tag="ht")
                            for jt in range(JT):
                                nc.tensor.matmul(
                                    ht[:],
                                    X[:, jt, dc * 128:(dc + 1) * 128],
                                    u[jt][:],
                                    start=(jt == 0),
                                    stop=(jt == JT - 1),
                                )
                            hst = hsp.tile([128, 512], F32R, tag="hs")
                            nc.vector.tensor_copy(hst[:], ht[:])
                            hs.append(hst)

                        # denominator -> per-partition reciprocal [128, 4]
                        dn_sb = smallp.tile([1, 512], F32, tag="dnsb")
                        nc.vector.tensor_copy(dn_sb[:], dn_ps[:])
                        dt_ps = ps_mm.tile([128, DC], F32, tag="mm")
                        for q in range(4):
                            nc.tensor.transpose(
                                dt_ps[:, q:q + 1],
                                dn_sb[:, q * 128:(q + 1) * 128],
                                ident[:1, :1],
                            )
                        recip = smallp.tile([128, 4], F32, tag="recip", bufs=2)
                        nc.vector.reciprocal(recip[:], dt_ps[:])

                        # y[i, f] += (headsT_un @ Wa_h) / denom
                        for q in range(4):
                            t = ib * 4 + q
                            yp = ps_mm.tile([128, 512], F32, tag="mm")
                            for dc in range(DC):
                                nc.tensor.matmul(
                                    yp[:],
                                    hs[dc][:, q * 128:(q + 1) * 128],
                                    wa_sb[:, dc, :],
                                    start=(dc == 0),
                                    stop=(dc == DC - 1),
                                )
                            if h == 0:
                                nc.vector.tensor_scalar_mul(
                                    y_acc[t][:], yp[:], recip[:, q:q + 1]
                                )
                            else:
                                ty = tmpp.tile([128, 512], F32, tag="ytmp")
                                nc.vector.tensor_scalar_mul(
                                    ty[:], yp[:], recip[:, q:q + 1]
                                )
                                nc.vector.tensor_add(
                                    y_acc[t][:], y_acc[t][:], ty[:]
                                )

                # y += b_att
                for t in range(T8):
                    nc.vector.tensor_add(y_acc[t][:], y_acc[t][:], ba_bc[:])

            # ---- stage D: feed-forward on my 1024 rows ----
            with (
                tc.tile_pool(name="ytp", bufs=DC) as ytp,
                tc.tile_pool(name="w1p", bufs=DC) as w1p,
                tc.tile_pool(name="w2p", bufs=1) as w2p,
                tc.tile_pool(name="ztp", bufs=GC) as ztp,
            ):
                w1_sb = [w1p.tile([128, DFF], F32R, tag="w1", name=f"w1c{d}")
                         for d in range(DC)]
                for fc in range(DC):
                    nc.sync.dma_start(
                        w1_sb[fc][:],
                        w1_d[fc * 128:(fc + 1) * 128, :].bitcast(F32R),
                    )
                b2_bc = w2p.tile([128, D], F32, tag="b2bc")
                nc.sync.dma_start(b2_bc[:], bcast_ap(b2_d[:]))
                w2_sb = w2p.tile([128, GC, D], F32R, tag="w2")
                nc.sync.dma_start(
                    w2_sb[:], w2_d.rearrange("(c p) e -> p c e", p=128).bitcast(F32R)
                )

                # yT[f, r] via PE transposes of y_acc
                yT = [ytp.tile([128, NQ], F32R, tag="yT", name=f"yT{d}")
                      for d in range(DC)]
                for t in range(T8):
                    for fc in range(DC):
                        tp = ps_mm.tile([128, 128], F32, tag="mm")
                        nc.tensor.transpose(
                            tp[:], y_acc[t][:, fc * 128:(fc + 1) * 128], ident[:]
                        )
                        nc.vector.tensor_copy(
                            yT[fc][:, t * 128:(t + 1) * 128], tp[:]
                        )

                # zT[g, r] = relu(W1^T y + b1)
                zT = [ztp.tile([128, NQ], F32R, tag="zT", name=f"zT{g}")
                      for g in range(GC)]
                for gc in range(GC):
                    for rb in range(NQ // 512):
                        zp = ps_mm.tile([128, 512], F32, tag="mm")
                        for fc in range(DC):
                            nc.tensor.matmul(
                                zp[:],
                                w1_sb[fc][:, gc * 128:(gc + 1) * 128],
                                yT[fc][:, rb * 512:(rb + 1) * 512],
                                start=(fc == 0),
                                stop=(fc == DC - 1),
                            )
                        nc.scalar.activation(
                            zT[gc][:, rb * 512:(rb + 1) * 512],
                            zp[:],
                            RELU,
                            bias=b1_t[:, gc:gc + 1],
                        )

                # out[r, e] = z @ W2 + b2
                for t in range(T8):
                    op = ps_mm.tile([128, 512], F32, tag="mm")
                    for gc in range(GC):
                        nc.tensor.matmul(
                            op[:],
                            zT[gc][:, t * 128:(t + 1) * 128],
                            w2_sb[:, gc, :],
                            start=(gc == 0),
                            stop=(gc == GC - 1),
                        )
                    ot = ztp.tile([128, 512], F32, tag="ot", bufs=2)
                    nc.vector.tensor_add(ot[:], op[:], b2_bc[:])
                    nc.sync.dma_start(out_d[t * 128:(t + 1) * 128, :], ot[:])

    return nc


_NC = None


def _get_nc():
    global _NC
    if _NC is None:
        _apply_patches()
        _NC = build_nc()
    return _NC


def make_in_maps(x, Wk, bk, Wq, bq, W_att, b_att, W1, b1, W2, b2):
    f = np.float32
    shared = {
        "wk": np.ascontiguousarray(Wk, f),
        "wq": np.ascontiguousarray(Wq, f),
        "bk": np.ascontiguousarray(bk, f),
        "bq": np.ascontiguousarray(bq, f),
        "watt": np.ascontiguousarray(np.asarray(W_att, f).reshape(H, D, D)),
        "batt": np.ascontiguousarray(b_att, f),
        "w1": np.ascontiguousarray(W1, f),
        "b1": np.ascontiguousarray(b1, f),
        "w2": np.ascontiguousarray(W2, f),
        "b2": np.ascontiguousarray(b2, f),
    }
    x = np.asarray(x, f)
    in_maps = []
    for c in range(8):
        b, half = divmod(c, 2)
        s = half * NQ
        xb = np.ascontiguousarray(np.concatenate([x[b, s:], x[b, :s]], axis=0))
        in_maps.append({"xb": xb, **shared})
    return in_maps


def assemble(results, dtype=np.float32):
    out = np.empty((B, N, D), dtype)
    for c in range(8):
        b, half = divmod(c, 2)
        out[b, half * NQ:(half + 1) * NQ] = results[c]["out"]
    return out


def kernel(**inputs):
    from concourse.bass_utils import run_bass_kernel_spmd

    nc = _get_nc()
    in_maps = make_in_maps(**inputs)
    res = run_bass_kernel_spmd(nc, in_maps, core_ids=list(range(8)))
    return assemble(res.results, dtype=np.asarray(inputs["x"]).dtype)


# revision 7
# speedup vs baseline: 198.0777x; 198.0777x over previous
"""Trainium2 Bass kernel for nn_NewSSHA_6528350290307.

Model (see reference): per-head K/Q projections over full d_model, unscaled
attention scores[i,j] = k_i . q_j softmaxed over j, heads = attn @ x, concat
heads -> att_linear -> relu MLP.

Sharding: 8 cores = 4 batches x 2 query-row halves. Each core owns 1024
output rows end-to-end: it computes q projections for its whole batch
(j side), k projections for its 1024 query rows, per-head attention, and
folds the head-concat @ W_att into a sum over heads of
head_h @ W_att[h*512:(h+1)*512], so no cross-core communication is needed.
The host rotates each batch so a core's query rows are always rows 0:1024
of its input (attention over keys j is permutation invariant).

Matmuls run as float32r (fp32 bits, PE rounds to ~13-bit mantissa,
1 cycle/row instead of fp32's 4). Transposes and the softmax denominator
pipeline stay exact fp32.
"""

import sys

sys.path.insert(0, "/opt/trn_rl_repo")

import numpy as np

import concourse.bass as bass
import concourse.mybir as mybir
import concourse.tile as tile

F32 = mybir.dt.float32
F32R = mybir.dt.float32r

B, N, D, H, DFF = 4, 2048, 512, 8, 2048
NQ = N // 2          # query rows per core
JT = N // 128        # 16 key tiles of 128 rows
IB = NQ // 512       # 2 i-blocks of 512 query rows
DC = D // 128        # 4 chunks of d_model
GC = DFF // 128      # 16 chunks of d_ff
T8 = NQ // 128       # 8 query chunks of 128
EXP = mybir.ActivationFunctionType.Exp
RELU = mybir.ActivationFunctionType.Relu

# ---------------------------------------------------------------------------
# Workarounds for the 1-sync-wait-per-instruction cap in this walrus build.
# ---------------------------------------------------------------------------


def _apply_patches():
    import re

    import orjson

    import concourse.bass2jax as bass2jax
    import concourse.bass_utils as bass_utils
    from concourse.tile_scheduler import is_hwdge_proc, is_swdge_proc

    if getattr(tile.TileContext, "_ssha_patched", False):
        return

    # 1. Tail drain: the stock exit stacks one wait per live proc sem on a
    #    single Drain instruction; split into individual wait_ge nops.
    def _drain_and_barrier(self, tick_clock, wait_clock):
        nc = self.nc
        gc = tick_clock.global_clock
        try:
            ticks = [gc[p] for p in range(27)]
        except TypeError:
            ticks = [int(s) for s in re.findall(r"\d+", repr(gc))]
        alloc = wait_clock.sems.allocated()
        for proc, sem in sorted(alloc.items()):
            t = ticks[proc] if proc < len(ticks) else 0
            if t <= 0:
                continue
            val = t * 16 if (is_swdge_proc(proc) or is_hwdge_proc(proc)) else t
            nc.sync.wait_ge(sem, val)
        nc.sync.drain()
        nc.all_engine_barrier()
        assert self.sems is not None
        popped = nc._tile_sem_poison_stack.pop()
        assert popped is self._sem_poison
        nc.clear_and_free_semaphores(list(self.sems.allocated().values()))
        nc.all_engine_barrier()

    tile.TileContext._drain_and_barrier = _drain_and_barrier
    tile.TileContext._ssha_patched = True

    # 2. Tile can still attach 2 waits to one instruction (e.g. self-loading
    #    fp32/f32r matmuls whose codegen has a single S3_LW sync slot).
    #    Post-process the BIR JSON: spill all but one wait per instruction
    #    into EventSemaphore instructions inserted right before it.
    engine_like = {"PE", "DVE", "Activation", "Pool", "SP"}
    counter = [0]

    def _cap_waits(bir_json):
        j = orjson.loads(bir_json)
        changed = False
        for fn in j.get("functions", []):
            for blk in fn.get("blocks", []):
                insts = blk.get("instructions")
                if not insts:
                    continue
                out = []
                for ins in insts:
                    si = ins.get("sync_info")
                    ow = (si or {}).get("on_wait") or []
                    if len(ow) > 1:
                        eng = ins.get("engine")
                        assert eng in engine_like, (
                            f"{ins.get('name')} on {eng} has {len(ow)} waits"
                        )
                        for w in ow[:-1]:
                            counter[0] += 1
                            out.append({
                                "debug": ins.get("debug", 0),
                                "engine": eng,
                                "ins": [],
                                "name": f"I-wspill-{counter[0]}",
                                "opcode": "EventSemaphore",
                                "outs": [],
                                "sync_info": {"on_update": [], "on_wait": [w]},
                            })
                        si["on_wait"] = ow[-1:]
                        changed = True
                    out.append(ins)
                blk["instructions"] = out
        return orjson.dumps(j) if changed else bir_json

    orig_compile = bass_utils.compile_bir_kernel

    def _patched_compile(bir_json, tmpdir, neff_name="file.neff"):
        if isinstance(bir_json, str):
            bir_json = bir_json.encode()
        return orig_compile(_cap_waits(bir_json), tmpdir, neff_name)

    bass_utils.compile_bir_kernel = _patched_compile
    bass2jax.compile_bir_kernel = _patched_compile


# ---------------------------------------------------------------------------
# Kernel builder
# ---------------------------------------------------------------------------


def build_nc(heads=H):
    from concourse.masks import make_identity

    nc = bass.Bass(dynamic_dma_scratch_size=2048)

    xb_d = nc.declare_dram_parameter("xb", [N, D], F32, isOutput=False)
    wk_d = nc.declare_dram_parameter("wk", [H, D, D], F32, isOutput=False)
    wq_d = nc.declare_dram_parameter("wq", [H, D, D], F32, isOutput=False)
    bk_d = nc.declare_dram_parameter("bk", [H, D], F32, isOutput=False)
    bq_d = nc.declare_dram_parameter("bq", [H, D], F32, isOutput=False)
    wa_d = nc.declare_dram_parameter("watt", [H, D, D], F32, isOutput=False)
    ba_d = nc.declare_dram_parameter("batt", [D], F32, isOutput=False)
    w1_d = nc.declare_dram_parameter("w1", [D, DFF], F32, isOutput=False)
    b1_d = nc.declare_dram_parameter("b1", [DFF], F32, isOutput=False)
    w2_d = nc.declare_dram_parameter("w2", [DFF, D], F32, isOutput=False)
    b2_d = nc.declare_dram_parameter("b2", [D], F32, isOutput=False)
    out_d = nc.declare_dram_parameter("out", [NQ, D], F32, isOutput=True)

    def bcast_ap(vec_ap, parts=128):
        # [n] dram vector -> [parts, n] with partition step 0
        return bass.AP(
            tensor=vec_ap.tensor,
            offset=vec_ap.offset,
            ap=[[0, parts]] + list(vec_ap.ap),
        )

    with tile.TileContext(nc) as tc:
        with (
            tc.tile_pool(name="const", bufs=1) as constp,
            tc.tile_pool(name="yacc", bufs=8) as yaccp,
            tc.tile_pool(name="small", bufs=1) as smallp,
            tc.tile_pool(name="tmp", bufs=2) as tmpp,
            tc.tile_pool(name="ps_mm", bufs=3, space="PSUM") as ps_mm,
        ):
            ident = constp.tile([128, 128], F32)
            make_identity(nc, ident)
            ones_f = constp.tile([128, 1], F32)
            nc.vector.memset(ones_f, 1.0)
            ones_r = constp.tile([128, 1], F32R)
            nc.vector.tensor_copy(ones_r[:], ones_f[:])
            # biases as per-partition columns
            bk_t = constp.tile([128, H, DC], F32)
            nc.sync.dma_start(bk_t[:], bk_d.rearrange("h (c p) -> p h c", p=128))
            bq_t = constp.tile([128, H, DC], F32)
            nc.sync.dma_start(bq_t[:], bq_d.rearrange("h (c p) -> p h c", p=128))
            b1_t = constp.tile([128, GC], F32)
            nc.sync.dma_start(b1_t[:], b1_d.rearrange("(c p) -> p c", p=128))
            # broadcast b_att / b2 across partitions
            ba_bc = constp.tile([128, D], F32)
            nc.sync.dma_start(ba_bc[:], bcast_ap(ba_d[:]))

            y_acc = [yaccp.tile([128, D], F32, tag="yacc", name=f"yacc{t}")
                      for t in range(T8)]

            with (
                tc.tile_pool(name="xp", bufs=1) as xp,
                tc.tile_pool(name="xtp", bufs=DC) as xtp,
                tc.tile_pool(name="qkp", bufs=DC) as qkp,
                tc.tile_pool(name="wp", bufs=2) as wp,
                tc.tile_pool(name="up", bufs=JT) as up,
                tc.tile_pool(name="hsp", bufs=DC) as hsp,
                tc.tile_pool(name="ps_sc", bufs=2, space="PSUM") as ps_sc,
                tc.tile_pool(name="ps_ht", bufs=2, space="PSUM") as ps_ht,
                tc.tile_pool(name="ps_dn", bufs=1, space="PSUM") as ps_dn,
            ):
                # ---- stage A: load x (rows layout, f32r bits) ----
                X = xp.tile([128, JT, D], F32R, tag="X")
                nc.sync.dma_start(
                    X[:], xb_d.rearrange("(rt p) d -> p rt d", p=128).bitcast(F32R)
                )

                # ---- stage B: xT[d, j] via PE transposes ----
                xT = [xtp.tile([128, N], F32R, tag="xT", name=f"xT{d}")
                      for d in range(DC)]
                for rt in range(JT):
                    for dc in range(DC):
                        tp = ps_mm.tile([128, 128], F32, tag="mm")
                        nc.tensor.transpose(
                            tp[:],
                            X[:, rt, dc * 128:(dc + 1) * 128].bitcast(F32),
                            ident[:],
                        )
                        nc.vector.tensor_copy(
                            xT[dc][:, rt * 128:(rt + 1) * 128], tp[:]
                        )

                # ---- stage C: heads ----
                for h in range(heads):
                    wk_sb = wp.tile([128, DC, D], F32R, tag="wkq")
                    nc.sync.dma_start(
                        wk_sb[:],
                        wk_d[h].rearrange("(c p) f -> p c f", p=128).bitcast(F32R),
                    )
                    wq_sb = wp.tile([128, DC, D], F32R, tag="wkq")
                    nc.sync.dma_start(
                        wq_sb[:],
                        wq_d[h].rearrange("(c p) f -> p c f", p=128).bitcast(F32R),
                    )
                    wa_sb = wp.tile([128, DC, D], F32R, tag="wa")
                    nc.sync.dma_start(
                        wa_sb[:],
                        wa_d[h].rearrange("(c p) f -> p c f", p=128).bitcast(F32R),
                    )

                    # projections: qT[f, j] (all j), kT[f, i] (my 1024 rows)
                    qT = [qkp.tile([128, N], F32R, tag="qT", name=f"qT{d}")
                          for d in range(DC)]
                    kT = [qkp.tile([128, NQ], F32R, tag="kT", name=f"kT{d}")
                          for d in range(DC)]
                    for fc in range(DC):
                        for jb in range(N // 512):
                            pp = ps_mm.tile([128, 512], F32, tag="mm")
                            for kc in range(DC):
                                nc.tensor.matmul(
                                    pp[:],
                                    wq_sb[:, kc, fc * 128:(fc + 1) * 128],
                                    xT[kc][:, jb * 512:(jb + 1) * 512],
                                    start=(kc == 0),
                                    stop=(kc == DC - 1),
                                )
                            nc.vector.tensor_scalar_add(
                                qT[fc][:, jb * 512:(jb + 1) * 512],
                                pp[:],
                                bq_t[:, h, fc:fc + 1],
                            )
                        for ibk in range(NQ // 512):
                            pp = ps_mm.tile([128, 512], F32, tag="mm")
                            for kc in range(DC):
                                nc.tensor.matmul(
                                    pp[:],
                                    wk_sb[:, kc, fc * 128:(fc + 1) * 128],
                                    xT[kc][:, ibk * 512:(ibk + 1) * 512],
                                    start=(kc == 0),
                                    stop=(kc == DC - 1),
                                )
                            nc.vector.tensor_scalar_add(
                                kT[fc][:, ibk * 512:(ibk + 1) * 512],
                                pp[:],
                                bk_t[:, h, fc:fc + 1],
                            )

                    for ib in range(IB):
                        isl = slice(ib * 512, (ib + 1) * 512)
                        # scores^T -> exp -> denominator, per key tile jt
                        u = []
                        dn_ps = ps_dn.tile([1, 512], F32, tag="dn")
                        sc_prev = None
                        for jt in range(JT):
                            sc = ps_sc.tile([128, 512], F32, tag="sc")
                            for fc in range(DC):
                                nc.tensor.matmul(
                                    sc[:],
                                    qT[fc][:, jt * 128:(jt + 1) * 128],
                                    kT[fc][:, isl],
                                    start=(fc == 0),
                                    stop=(fc == DC - 1),
                                )
                            ut = up.tile([128, 512], F32R, tag="u")
                            nc.scalar.activation(ut[:], sc[:], EXP)
                            u.append(ut)
                            # stagger the denominator matmul one tile behind so
                            # the PE never waits on the exp it just requested
                            if jt >= 1:
                                nc.tensor.matmul(
                                    dn_ps[:], ones_r[:], u[jt - 1][:],
                                    start=(jt == 1), stop=False,
                                )
                        nc.tensor.matmul(
                            dn_ps[:], ones_r[:], u[JT - 1][:],
                            start=False, stop=True,
                        )

                        # attn @ x : headsT_un[d, i] accumulated over j
                        hs = []
                        for dc in range(DC):
                            ht = ps_ht.tile([128, 512], F32, tag="ht")
                            for jt in range(JT):
                                nc.tensor.matmul(
                                    ht[:],
                                    X[:, jt, dc * 128:(dc + 1) * 128],
                                    u[jt][:],
                                    start=(jt == 0),
                                    stop=(jt == JT - 1),
                                )
                            hst = hsp.tile([128, 512], F32R, tag="hs")
                            nc.vector.tensor_copy(hst[:], ht[:])
                            hs.append(hst)

                        # denominator -> per-partition reciprocal [128, 4]
                        dn_sb = smallp.tile([1, 512], F32, tag="dnsb")
                        nc.vector.tensor_copy(dn_sb[:], dn_ps[:])
                        dt_ps = ps_mm.tile([128, DC], F32, tag="mm")
                        for q in range(4):
                            nc.tensor.transpose(
                                dt_ps[:, q:q + 1],
                                dn_sb[:, q * 128:(q + 1) * 128],
                                ident[:1, :1],
                            )
                        recip = smallp.tile([128, 4], F32, tag="recip", bufs=2)
                        nc.vector.reciprocal(recip[:], dt_ps[:])

                        # y[i, f] += (headsT_un @ Wa_h) / denom
                        for q in range(4):
                            t = ib * 4 + q
                            yp = ps_mm.tile([128, 512], F32, tag="mm")
                            for dc in range(DC):
                                nc.tensor.matmul(
                                    yp[:],
                                    hs[dc][:, q * 128:(q + 1) * 128],
                                    wa_sb[:, dc, :],
                                    start=(dc == 0),
                                    stop=(dc == DC - 1),
                                )
                            if h == 0:
                                nc.vector.tensor_scalar_mul(
                                    y_acc[t][:], yp[:], recip[:, q:q + 1]
                                )
                            else:
                                ty = tmpp.tile([128, 512], F32, tag="ytmp")
                                nc.vector.tensor_scalar_mul(
                                    ty[:], yp[:], recip[:, q:q + 1]
                                )
                                nc.vector.tensor_add(
                                    y_acc[t][:], y_acc[t][:], ty[:]
                                )

                # y += b_att
                for t in range(T8):
                    nc.vector.tensor_add(y_acc[t][:], y_acc[t][:], ba_bc[:])

            # ---- stage D: feed-forward on my 1024 rows ----
            with (
                tc.tile_pool(name="ytp", bufs=DC) as ytp,
                tc.tile_pool(name="w1p", bufs=DC) as w1p,
                tc.tile_pool(name="w2p", bufs=1) as w2p,
                tc.tile_pool(name="ztp", bufs=GC) as ztp,
            ):
                w1_sb = [w1p.tile([128, DFF], F32R, tag="w1", name=f"w1c{d}")
                         for d in range(DC)]
                for fc in range(DC):
                    nc.sync.dma_start(
                        w1_sb[fc][:],
                        w1_d[fc * 128:(fc + 1) * 128, :].bitcast(F32R),
                    )
                b2_bc = w2p.tile([128, D], F32, tag="b2bc")
                nc.sync.dma_start(b2_bc[:], bcast_ap(b2_d[:]))
                w2_sb = w2p.tile([128, GC, D], F32R, tag="w2")
                nc.sync.dma_start(
                    w2_sb[:], w2_d.rearrange("(c p) e -> p c e", p=128).bitcast(F32R)
                )

                # yT[f, r] via PE transposes of y_acc
                yT = [ytp.tile([128, NQ], F32R, tag="yT", name=f"yT{d}")
                      for d in range(DC)]
                for t in range(T8):
                    for fc in range(DC):
                        tp = ps_mm.tile([128, 128], F32, tag="mm")
                        nc.tensor.transpose(
                            tp[:], y_acc[t][:, fc * 128:(fc + 1) * 128], ident[:]
                        )
                        nc.vector.tensor_copy(
                            yT[fc][:, t * 128:(t + 1) * 128], tp[:]
                        )

                # zT[g, r] = relu(W1^T y + b1)
                zT = [ztp.tile([128, NQ], F32R, tag="zT", name=f"zT{g}")
                      for g in range(GC)]
                for gc in range(GC):
                    for rb in range(NQ // 512):
                        zp = ps_mm.tile([128, 512], F32, tag="mm")
                        for fc in range(DC):
                            nc.tensor.matmul(
                                zp[:],
                                w1_sb[fc][:, gc * 128:(gc + 1) * 128],
                                yT[fc][:, rb * 512:(rb + 1) * 512],
                                start=(fc == 0),
                                stop=(fc == DC - 1),
                            )
                        nc.scalar.activation(
                            zT[gc][:, rb * 512:(rb + 1) * 512],
                            zp[:],
                            RELU,
                            bias=b1_t[:, gc:gc + 1],
                        )

                # out[r, e] = z @ W2 + b2
                for t in range(T8):
                    op = ps_mm.tile([128, 512], F32, tag="mm")
                    for gc in range(GC):
                        nc.tensor.matmul(
                            op[:],
                            zT[gc][:, t * 128:(t + 1) * 128],
                            w2_sb[:, gc, :],
                            start=(gc == 0),
                            stop=(gc == GC - 1),
                        )
                    ot = ztp.tile([128, 512], F32, tag="ot", bufs=2)
                    nc.vector.tensor_add(ot[:], op[:], b2_bc[:])
                    nc.sync.dma_start(out_d[t * 128:(t + 1) * 128, :], ot[:])

    return nc


_NC = None


def _get_nc():
    global _NC
    if _NC is None:
        _apply_patches()
        _NC = build_nc()
    return _NC


def make_in_maps(x, Wk, bk, Wq, bq, W_att, b_att, W1, b1, W2, b2):
    f = np.float32
    shared = {
        "wk": np.ascontiguousarray(Wk, f),
        "wq": np.ascontiguousarray(Wq, f),
        "bk": np.ascontiguousarray(bk, f),
        "bq": np.ascontiguousarray(bq, f),
        "watt": np.ascontiguousarray(np.asarray(W_att, f).reshape(H, D, D)),
        "batt": np.ascontiguousarray(b_att, f),
        "w1": np.ascontiguousarray(W1, f),
        "b1": np.ascontiguousarray(b1, f),
        "w2": np.ascontiguousarray(W2, f),
        "b2": np.ascontiguousarray(b2, f),
    }
    x = np.asarray(x, f)
    in_maps = []
    for c in range(8):
        b, half = divmod(c, 2)
        s = half * NQ
        xb = np.ascontiguousarray(np.concatenate([x[b, s:], x[b, :s]], axis=0))
        in_maps.append({"xb": xb, **shared})
    return in_maps


def assemble(results, dtype=np.float32):
    out = np.empty((B, N, D), dtype)
    for c in range(8):
        b, half = divmod(c, 2)
        out[b, half * NQ:(half + 1) * NQ] = results[c]["out"]
    return out


_RUNNER = None


def _build_runner():
    """jit(shard_map) over the 8 cores with weights replicated (shipped once
    over the axon tunnel) and only xb/out sharded per-core."""
    import jax
    from jax.sharding import Mesh, PartitionSpec
    from jax.experimental.shard_map import shard_map

    import concourse.bass2jax as b2j

    nc = _get_nc()
    b2j.install_neuronx_cc_hook()
    partition_name = nc.partition_id_tensor.name if nc.partition_id_tensor else None
    in_names, out_names, out_shapes = [], [], []
    for alloc in nc.m.functions[0].allocations:
        if not isinstance(alloc, mybir.MemoryLocationSet):
            continue
        name = alloc.memorylocations[0].name
        if alloc.kind == "ExternalInput":
            if name != partition_name:
                in_names.append(name)
        elif alloc.kind == "ExternalOutput":
            out_names.append(name)
            out_shapes.append(
                (tuple(alloc.tensor_shape), mybir.dt.np(alloc.dtype))
            )
    import jax.core as jcore

    out_avals = tuple(jcore.ShapedArray(s, d) for s, d in out_shapes)
    all_in_names = list(in_names) + list(out_names)
    if partition_name is not None:
        all_in_names.append(partition_name)

    def _body(*args):
        operands = list(args)
        if partition_name is not None:
            operands.append(b2j.partition_id_tensor())
        outs = b2j._bass_exec_p.bind(
            *operands,
            out_avals=out_avals,
            in_names=tuple(all_in_names),
            out_names=tuple(out_names),
            lowering_input_output_aliases=(),
            sim_require_finite=True,
            sim_require_nnan=True,
            nc=nc,
        )
        return tuple(outs)

    devices = jax.devices()[:8]
    mesh = Mesh(np.asarray(devices), ("core",))
    sharded_names = {"xb"}
    in_specs = tuple(
        PartitionSpec("core") if n in sharded_names else PartitionSpec()
        for n in in_names
    ) + (PartitionSpec("core"),) * len(out_names)
    out_specs = (PartitionSpec("core"),) * len(out_names)
    n_params = len(in_names)
    donate = tuple(range(n_params, n_params + len(out_names)))
    fn = jax.jit(
        shard_map(_body, mesh=mesh, in_specs=in_specs, out_specs=out_specs,
                  check_rep=False),
        donate_argnums=donate,
        keep_unused=True,
    )
    return fn, in_names, out_names, out_shapes, sharded_names


def _get_runner():
    global _RUNNER
    if _RUNNER is None:
        _RUNNER = _build_runner()
    return _RUNNER


def _run_custom(in_maps):
    import jax

    fn, in_names, out_names, out_shapes, sharded_names = _get_runner()
    args = []
    for n in in_names:
        if n in sharded_names:
            args.append(np.concatenate(
                [np.asarray(in_maps[c][n]) for c in range(8)], axis=0))
        else:
            args.append(np.asarray(in_maps[0][n]))
    for shape, dt in out_shapes:
        args.append(np.zeros((8 * shape[0], *shape[1:]), dt))
    outs = fn(*args)
    jax.block_until_ready(outs)
    return [
        {n: np.asarray(outs[i]).reshape(8, *out_shapes[i][0])[c]
         for i, n in enumerate(out_names)}
        for c in range(8)
    ]


def kernel(**inputs):
    nc = _get_nc()
    in_maps = make_in_maps(**inputs)
    results = None
    try:
        results = _run_custom(in_maps)
    except Exception:
        # fall back to the stock SPMD runner (ships weights per-core)
        from concourse.bass_utils import run_bass_kernel_spmd

        results = run_bass_kernel_spmd(
            nc, in_maps, core_ids=list(range(8))
        ).results
    return assemble(results, dtype=np.asarray(inputs["x"]).dtype)
